# revision 37
# baseline (speedup 1.0000x reference)
"""Trainium2 Bass kernel for nn_AttentionLayer (B=8, N=1024, D=256, H=4).

Sharding: pure data-parallel over batch B across 8 NeuronCores (one batch
element per core, all parameters replicated). No collectives.

The host wrapper additionally gathers the unmasked keys (mask is ~Bernoulli(.5))
into a compact x_keys [KP=640, D] tensor, so the attention contracts over 5
key chunks instead of 8 -- exactly equivalent because masked keys contribute
nothing (V rows are zeroed by the key mask and the denominator is mask-weighted).

Per-core algorithm (bf16 matmuls, fp32 stats/output, all free-dim layouts):
  x_norm = LN(x); LN(x_keys)                   (bn_stats; gamma/beta folded
                                                into the transpose copies)
  xnT, xnkT via tensor-engine transposes       (no DMA-xbar mode switches)
  per head h:
    q^T = wq-chunk.T @ xnT;  k^T over keys     (weights stay natural layout)
    s^T  = kT-chunk.T @ qT                     ([key-chunk, n] logits in PSUM)
    esT  = exp(s^T/16)                         (ACT; already the av lhsT layout)
    v''  = [(xnk @ wv_h) * m_k | m_k]          ([m, 257], key mask folded)
    out  = esT.T @ v''                         ([n, 257]; col 256 = denominator)
    gate = sigmoid via tanh                    (tanh shares the exp ACT table)
    t_h  = out * gate * (0.5*m_q/denom) + x    (fused scalar_tensor_tensor)
  z    = concat_h LN_lnr(t_h)    (gamma folded into out_w; beta via bias row)
  y    = zT.T @ out_w' + (out_b + beta_r@out_w) + x   (zT via DMA-xbar)
  out  = LN_lno(y) * mask
Weights stream in via SWDGE cast-DMAs (f32->bf16 during transfer); ~72 dummy
matmuls at start trip the PE HAM clock gate to full rate before the real work.
Measured ~144 us/core on TRN2, rel err vs the fp32 reference ~2.9e-3.
"""

import os
import sys

for _p in ("/opt/trn_rl_repo", "/root/.axon_site/_ro/trn_rl_repo"):
    if os.path.isdir(_p) and _p not in sys.path:
        sys.path.insert(0, _p)
        break

import numpy as np

N, D, H = 1024, 256, 4
NCH = N // 128  # 8 token chunks
KP = 640  # padded count of unmasked keys (max over batches is ~547)
KCH = KP // 128
EPS = 1e-6
SCALE = 1.0 / 16.0

_PROGRAM = None  # built Bass program, cached across kernel() calls


def _build_program():
    from contextlib import ExitStack

    import concourse.bass as bass
    import concourse.mybir as mybir
    import concourse.tile as tile
    from concourse import bacc
    from concourse.masks import make_identity

    f32 = mybir.dt.float32
    bf16 = mybir.dt.bfloat16
    i32 = mybir.dt.int32
    AF = mybir.ActivationFunctionType
    OP = mybir.AluOpType

    nc = bacc.Bacc(
        "TRN2",
        target_bir_lowering=False,
        debug=False,
        enable_asserts=False,
        num_devices=8,
    )

    x_d = nc.dram_tensor("x", [N, D], f32, kind="ExternalInput")
    mask_d = nc.dram_tensor("mask", [N], i32, kind="ExternalInput")
    xk_d = nc.dram_tensor("x_keys", [KP, D], f32, kind="ExternalInput")
    mk_d = nc.dram_tensor("mask_keys", [KP], i32, kind="ExternalInput")
    wq_d = nc.dram_tensor("wq", [H, D, D], f32, kind="ExternalInput")
    wk_d = nc.dram_tensor("wk", [H, D, D], f32, kind="ExternalInput")
    wv_d = nc.dram_tensor("wv", [H, D, D], f32, kind="ExternalInput")
    wg_d = nc.dram_tensor("wg", [H, D, D], f32, kind="ExternalInput")
    ow_d = nc.dram_tensor("out_w", [D * H, D], f32, kind="ExternalInput")
    ob_d = nc.dram_tensor("out_b", [D], f32, kind="ExternalInput")
    lng_d = nc.dram_tensor("ln_g", [D], f32, kind="ExternalInput")
    lnb_d = nc.dram_tensor("ln_b", [D], f32, kind="ExternalInput")
    lnrg_d = nc.dram_tensor("lnr_g", [D], f32, kind="ExternalInput")
    lnrb_d = nc.dram_tensor("lnr_b", [D], f32, kind="ExternalInput")
    lnog_d = nc.dram_tensor("lno_g", [D], f32, kind="ExternalInput")
    lnob_d = nc.dram_tensor("lno_b", [D], f32, kind="ExternalInput")
    y_d = nc.dram_tensor("y", [N, D], f32, kind="ExternalOutput")

    def bcast_ap(ap, parts=128):
        return bass.AP(
            tensor=ap.tensor, offset=ap.offset, ap=[[0, parts]] + list(ap.ap)
        )

    with tile.TileContext(nc) as tc, ExitStack() as ctx:
        const = ctx.enter_context(tc.tile_pool(name="const", bufs=1))
        big = ctx.enter_context(tc.tile_pool(name="big", bufs=1))
        hpool = ctx.enter_context(tc.tile_pool(name="hpool", bufs=2))
        spool = ctx.enter_context(tc.tile_pool(name="spool", bufs=8))
        small = ctx.enter_context(tc.tile_pool(name="small", bufs=3))
        ps_s = ctx.enter_context(tc.tile_pool(name="ps_s", bufs=2, space="PSUM"))
        ps_o = ctx.enter_context(tc.tile_pool(name="ps_o", bufs=2, space="PSUM"))
        ps_vg = ctx.enter_context(tc.tile_pool(name="ps_vg", bufs=2, space="PSUM"))

        # ---- stage 0a: x / mask on the sync ring (per chunk: LN starts early)
        # identity first on Q7 (gates the xn transposes), then x on the sync
        # HWDGE ring (descriptor gen is immediate there, so x beats the
        # SWDGE weight traffic to HBM), weights behind on SWDGE
        ident = const.tile([128, 128], bf16)
        make_identity(nc, ident)
        x_sb = const.tile([128, NCH, D], f32)
        nc.sync.dma_start(out=x_sb, in_=x_d.ap().rearrange("(c p) d -> p c d", p=128))
        xk_sb = const.tile([128, KCH, D], f32)
        nc.sync.dma_start(out=xk_sb, in_=xk_d.ap().rearrange("(c p) d -> p c d", p=128))
        mask_i = const.tile([128, NCH], i32)
        nc.sync.dma_start(out=mask_i, in_=mask_d.ap().rearrange("(c p) -> p c", p=128))
        mk_i = const.tile([128, KCH], i32)
        nc.sync.dma_start(out=mk_i, in_=mk_d.ap().rearrange("(c p) -> p c", p=128))
        wq_bf = const.tile([128, H, 2, D], bf16)
        wk_bf = const.tile([128, H, 2, D], bf16)
        wv_bf = const.tile([128, H, 2, D], bf16)
        wg_bf = const.tile([128, H, 2, D], bf16)
        for wd, wb_dst in ((wq_d, wq_bf), (wk_d, wk_bf), (wv_d, wv_bf), (wg_d, wg_bf)):
            nc.gpsimd.dma_start(
                out=wb_dst,
                in_=wd.ap().rearrange("h (c p) e -> p h c e", p=128),
            )

        # dummy matmuls to trip the PE HAM clock-gate to 8/8 before the real
        # stream begins (PE would otherwise sit cold through the LN ramp)
        warm_sink = const.tile([128, 128], f32)
        warm_ps = ps_o.tile([128, D + 1], f32, tag="o")
        for i in range(72):
            nc.tensor.matmul(
                warm_ps[:, 0:128], lhsT=ident, rhs=ident,
                start=(i == 0), stop=(i == 71),
            )
        nc.any.tensor_copy(out=warm_sink, in_=warm_ps[:, 0:128])

        eps_t = const.tile([128, 1], f32)
        nc.vector.memset(eps_t, EPS)
        zero_t = const.tile([128, 1], f32)
        nc.vector.memset(zero_t, 0.0)

        lng_col = const.tile([128, 2], f32)
        nc.scalar.dma_start(out=lng_col, in_=lng_d.ap().rearrange("(b p) -> p b", p=128))
        lnb_col = const.tile([128, 2], f32)
        nc.scalar.dma_start(out=lnb_col, in_=lnb_d.ap().rearrange("(b p) -> p b", p=128))

        # ---- stage 1: first layernorm + xnT (fully per-chunk pipelined,
        # transposes on the tensor engine: no DMA-xbar mode switches)
        xn_full = big.tile([128, NCH, D * H], bf16, tag="xz")
        xn = xn_full[:, :, 0:D]
        xnT = const.tile([128, 2, N], bf16)  # [p, dc, n] = xn^T[128*dc+p, n]
        x_bf = const.tile([128, NCH, D], bf16)
        for c in range(NCH):
            st6 = small.tile([128, 6], f32, tag="st6")
            nc.vector.bn_stats(out=st6, in_=x_sb[:, c, :])
            mv = small.tile([128, 2], f32, tag="mv")
            nc.vector.bn_aggr(out=mv, in_=st6)
            rs = small.tile([128, 1], f32, tag="rs")
            nc.scalar.activation(
                out=rs, in_=mv[:, 1:2], func=AF.Sqrt, bias=eps_t[:], scale=1.0
            )
            nc.vector.reciprocal(rs, rs)
            nc.vector.tensor_scalar(
                xn[:, c, :], x_sb[:, c, :], mv[:, 0:1], rs, OP.subtract, OP.mult
            )
            for dc in range(2):
                tr_ps = ps_vg.tile([128, 512], bf16, tag="pvg")
                nc.tensor.transpose(
                    tr_ps[:, 0:128], xn[:, c, 128 * dc : 128 * dc + 128], ident
                )
                # gamma/beta land here: after the transpose d is the
                # partition dim, so they are plain per-partition scalars
                nc.any.tensor_scalar(
                    xnT[:, dc, 128 * c : 128 * c + 128],
                    tr_ps[:, 0:128],
                    lng_col[:, dc : dc + 1],
                    lnb_col[:, dc : dc + 1],
                    OP.mult,
                    OP.add,
                )
            nc.any.tensor_copy(out=x_bf[:, c, :], in_=x_sb[:, c, :])

        xnk = big.tile([128, KCH, D], bf16)
        xnkT = const.tile([128, 2, KP], bf16)  # [p, dc, m] = xnk^T
        for c in range(KCH):
            st6 = small.tile([128, 6], f32, tag="st6")
            nc.vector.bn_stats(out=st6, in_=xk_sb[:, c, :])
            mv = small.tile([128, 2], f32, tag="mv")
            nc.vector.bn_aggr(out=mv, in_=st6)
            rs = small.tile([128, 1], f32, tag="rs")
            nc.scalar.activation(
                out=rs, in_=mv[:, 1:2], func=AF.Sqrt, bias=eps_t[:], scale=1.0
            )
            nc.vector.reciprocal(rs, rs)
            nc.vector.tensor_scalar(
                xnk[:, c, :], xk_sb[:, c, :], mv[:, 0:1], rs, OP.subtract, OP.mult
            )
            for dc in range(2):
                tr_ps = ps_vg.tile([128, 512], bf16, tag="pvg")
                nc.tensor.transpose(
                    tr_ps[:, 0:128], xnk[:, c, 128 * dc : 128 * dc + 128], ident
                )
                nc.any.tensor_scalar(
                    xnkT[:, dc, 128 * c : 128 * c + 128],
                    tr_ps[:, 0:128],
                    lng_col[:, dc : dc + 1],
                    lnb_col[:, dc : dc + 1],
                    OP.mult,
                    OP.add,
                )

        mask_f = const.tile([128, NCH], f32)
        nc.vector.tensor_copy(out=mask_f, in_=mask_i)
        m_half = const.tile([128, NCH], f32)
        nc.vector.tensor_scalar_mul(m_half, mask_f, 0.5)
        mk_f = const.tile([128, KCH], f32)
        nc.vector.tensor_copy(out=mk_f, in_=mk_i)
        mk_bf = const.tile([128, KCH], bf16)
        nc.any.tensor_copy(out=mk_bf, in_=mk_f)


        # ---- stage 2: heads
        t_all = big.tile([128, H, NCH, D], bf16, tag="tz")
        mv_r = big.tile([128, H, NCH, 2], f32)
        z = big.tile([128, NCH, D * H], bf16, tag="xz")  # [p(n), c, h*256+e]
        zT = big.tile([128, NCH, N], bf16)  # [p, kc, n] = z^T[128*kc+p, n]
        y_sb = big.tile([128, NCH, D], f32)
        y_out = big.tile([128, NCH, D], f32)

        def tail_prep_chunk(c):
            # lnr-normalize + transpose for one token chunk; called inside
            # head 3's av loop so this DVE/DMA work hides under av matmuls
            rs4 = small.tile([128, 4], f32, tag="rs4")
            nc.scalar.activation(
                out=rs4, in_=mv_r[:, :, c, 1], func=AF.Sqrt, bias=eps_t[:], scale=1.0
            )
            nc.vector.reciprocal(rs4, rs4)
            for h in range(H):
                nc.any.tensor_scalar(
                    z[:, c, D * h : D * (h + 1)],
                    t_all[:, h, c, :],
                    mv_r[:, h, c, 0:1],
                    rs4[:, h : h + 1],
                    OP.subtract,
                    OP.mult,
                )
            eng = nc.sync if c % 2 == 0 else nc.scalar
            eng.dma_start_transpose(
                out=zT[:, :, 128 * c : 128 * c + 128], in_=z[:, c, :]
            )

        def y_chunk(c):
            # final projection + residual + lno for one token chunk
            y_ps = ps_s.tile([128, D], f32, tag="s")
            for kc in range(NCH):
                nc.tensor.matmul(
                    y_ps,
                    lhsT=zT[:, kc, 128 * c : 128 * c + 128],
                    rhs=wo_bf[:, kc // 2, kc % 2, :],
                    start=(kc == 0),
                    stop=(kc == NCH - 1),
                )
            nc.any.tensor_add(y_sb[:, c, :], y_ps, xb[:, c, :])
            st6 = small.tile([128, 6], f32, tag="st6")
            nc.vector.bn_stats(out=st6, in_=y_sb[:, c, :])
            mvo = small.tile([128, 2], f32, tag="mv")
            nc.vector.bn_aggr(out=mvo, in_=st6)
            rso = small.tile([128, 1], f32, tag="rs")
            nc.scalar.activation(
                out=rso, in_=mvo[:, 1:2], func=AF.Sqrt, bias=eps_t[:], scale=1.0
            )
            nc.vector.reciprocal(rso, rso)
            f1 = small.tile([128, D], f32, tag="f1")
            nc.vector.scalar_tensor_tensor(
                out=f1, in0=y_sb[:, c, :], scalar=mvo[:, 0:1], in1=lnog_bc,
                op0=OP.subtract, op1=OP.mult,
            )
            f2 = small.tile([128, D], f32, tag="f2")
            nc.vector.scalar_tensor_tensor(
                out=f2, in0=f1, scalar=rso, in1=lnob_bc,
                op0=OP.mult, op1=OP.add,
            )
            nc.any.tensor_scalar(
                y_out[:, c, :], f2, mask_f[:, c : c + 1], None, OP.mult
            )
            nc.sync.dma_start(
                out=y_d.ap()[128 * c : 128 * (c + 1), :], in_=y_out[:, c, :]
            )

        for h in range(H):
            # q^T, k^T = [e, n] projections (weights stay natural: no
            # weight transposes needed)
            qT_bf = hpool.tile([128, 2, N], bf16, tag="qT")
            kT_bf = hpool.tile([128, 2, KP], bf16, tag="kT")
            for ec in range(2):
                for nh in range(2):
                    p_ps = ps_vg.tile([128, 512], f32, tag="pvg")
                    for kd in range(2):
                        nc.tensor.matmul(
                            p_ps,
                            lhsT=wq_bf[:, h, kd, 128 * ec : 128 * ec + 128],
                            rhs=xnT[:, kd, 512 * nh : 512 * nh + 512],
                            start=(kd == 0),
                            stop=(kd == 1),
                        )
                    nc.any.tensor_copy(
                        out=qT_bf[:, ec, 512 * nh : 512 * nh + 512], in_=p_ps
                    )
            for ec in range(2):
                for m0, mw in ((0, 512), (512, KP - 512)):
                    p_ps = ps_vg.tile([128, 512], f32, tag="pvg")
                    for kd in range(2):
                        nc.tensor.matmul(
                            p_ps[:, 0:mw],
                            lhsT=wk_bf[:, h, kd, 128 * ec : 128 * ec + 128],
                            rhs=xnkT[:, kd, m0 : m0 + mw],
                            start=(kd == 0),
                            stop=(kd == 1),
                        )
                    nc.any.tensor_copy(
                        out=kT_bf[:, ec, m0 : m0 + mw], in_=p_ps[:, 0:mw]
                    )

            # v'' = [xn @ wv * m_k | m_k]
            v2 = hpool.tile([128, KCH, D + 1], bf16, tag="v2")
            for mc in range(KCH):
                v_ps = ps_vg.tile([128, 512], f32, tag="pvg")
                for kd in range(2):
                    nc.tensor.matmul(
                        v_ps[:, 0:D],
                        lhsT=xnkT[:, kd, 128 * mc : 128 * mc + 128],
                        rhs=wv_bf[:, h, kd, :],
                        start=(kd == 0),
                        stop=(kd == 1),
                    )
                nc.any.tensor_scalar(
                    v2[:, mc, 0:D], v_ps[:, 0:D], mk_f[:, mc : mc + 1], None, OP.mult
                )
            nc.any.tensor_copy(out=v2[:, :, D], in_=mk_bf)

            # gate pre-activation: tanh(0.5 * xn @ wg)
            tanh_o = hpool.tile([128, NCH, D], bf16, tag="tanh")
            for c in range(NCH):
                g_ps = ps_vg.tile([128, 512], f32, tag="pvg")
                for kd in range(2):
                    nc.tensor.matmul(
                        g_ps[:, 0:D],
                        lhsT=xnT[:, kd, 128 * c : 128 * c + 128],
                        rhs=wg_bf[:, h, kd, :],
                        start=(kd == 0),
                        stop=(kd == 1),
                    )
                nc.scalar.activation(
                    out=tanh_o[:, c, :], in_=g_ps[:, 0:D], func=AF.Tanh,
                    bias=zero_t[:], scale=0.5,
                )

            # logits transposed: s^T tiles [m-chunk, n]; exp output is the
            # av lhsT layout directly (no transpose)
            esT_tiles = []
            for mc in range(KCH):
                s_ps = ps_s.tile([128, N], f32, tag="s")
                for kc in range(2):
                    for nh in range(2):
                        nc.tensor.matmul(
                            s_ps[:, 512 * nh : 512 * nh + 512],
                            lhsT=kT_bf[:, kc, 128 * mc : 128 * mc + 128],
                            rhs=qT_bf[:, kc, 512 * nh : 512 * nh + 512],
                            start=(kc == 0),
                            stop=(kc == 1),
                        )
                esT = spool.tile([128, N], bf16, tag="esT")
                nc.scalar.activation(
                    out=esT, in_=s_ps, func=AF.Exp, bias=zero_t[:], scale=SCALE
                )
                esT_tiles.append(esT)

            for c in range(NCH):
                o_ps = ps_o.tile([128, D + 1], f32, tag="o")
                for mc in range(KCH):
                    nc.tensor.matmul(
                        o_ps,
                        lhsT=esT_tiles[mc][:, 128 * c : 128 * c + 128],
                        rhs=v2[:, mc, :],
                        start=(mc == 0),
                        stop=(mc == KCH - 1),
                    )
                hf = small.tile([128, 1], f32, tag="hf")
                nc.vector.reciprocal(hf, o_ps[:, D : D + 1])
                nc.vector.tensor_scalar_mul(hf, hf, m_half[:, c : c + 1])
                tmp = small.tile([128, D], bf16, tag="tmp")
                nc.vector.scalar_tensor_tensor(
                    out=tmp,
                    in0=tanh_o[:, c, :],
                    scalar=1.0,
                    in1=o_ps[:, 0:D],
                    op0=OP.add,
                    op1=OP.mult,
                )
                nc.vector.scalar_tensor_tensor(
                    out=t_all[:, h, c, :],
                    in0=tmp,
                    scalar=hf,
                    in1=x_bf[:, c, :],
                    op0=OP.mult,
                    op1=OP.add,
                )
                st6 = small.tile([128, 6], f32, tag="st6")
                nc.vector.bn_stats(out=st6, in_=t_all[:, h, c, :])
                nc.vector.bn_aggr(out=mv_r[:, h, c, :], in_=st6)
                if h == H - 1:
                    tail_prep_chunk(c)

            if h == 1:
                # out_w / bias prep emitted mid-kernel: DMAs overlap head
                # compute, results only needed at the tail
                gcol = const.tile([128, 2], f32)
                nc.gpsimd.dma_start(
                    out=gcol, in_=lnrg_d.ap().rearrange("(b p) -> p b", p=128)
                )
                bcol_bf = const.tile([128, 2], bf16)
                nc.gpsimd.dma_start(
                    out=bcol_bf, in_=lnrb_d.ap().rearrange("(b p) -> p b", p=128)
                )
                # out_w permuted to [p, h, b, col] (row (128b+p)*4+h), bf16 cast
                wo_raw = const.tile([128, H, 2, D], bf16)
                nc.gpsimd.dma_start(
                    out=wo_raw,
                    in_=ow_d.ap().rearrange("(b p h) o -> p h b o", b=2, p=128, h=H),
                )
                wo_bf = const.tile([128, H, 2, D], bf16)
                for hh in range(H):
                    for b2 in range(2):
                        nc.any.tensor_scalar(
                            wo_bf[:, hh, b2, :],
                            wo_raw[:, hh, b2, :],
                            gcol[:, b2 : b2 + 1],
                            None,
                            OP.mult,
                        )
                ob_row = const.tile([1, D], f32)
                ob_ap = ob_d.ap()
                nc.gpsimd.dma_start(
                    out=ob_row,
                    in_=bass.AP(
                        tensor=ob_ap.tensor, offset=ob_ap.offset,
                        ap=[[0, 1]] + list(ob_ap.ap),
                    ),
                )
                lnog_bc = const.tile([128, D], f32)
                nc.gpsimd.dma_start(out=lnog_bc, in_=bcast_ap(lnog_d.ap()))
                lnob_bc = const.tile([128, D], f32)
                nc.gpsimd.dma_start(out=lnob_bc, in_=bcast_ap(lnob_d.ap()))

            if h == 2:
                # bias row = out_b + lnr_b @ out_w, broadcast via DRAM
                bias_ps = ps_o.tile([1, D], f32, tag="o")
                i = 0
                for b2 in range(2):
                    for hh in range(H):
                        nc.tensor.matmul(
                            bias_ps,
                            lhsT=bcol_bf[:, b2 : b2 + 1],
                            rhs=wo_raw[:, hh, b2, :],
                            start=(i == 0),
                            stop=(i == 7),
                        )
                        i += 1
                bias_row = const.tile([1, D], f32)
                nc.vector.tensor_add(bias_row, bias_ps, ob_row)
                bias_dram = nc.dram_tensor("bias_scratch", [D], f32, kind="Internal")
                nc.gpsimd.dma_start(
                    out=bias_dram.ap().rearrange("(o d) -> o d", o=1), in_=bias_row
                )
                bias_bc = const.tile([128, D], f32)
                nc.gpsimd.dma_start(out=bias_bc, in_=bcast_ap(bias_dram.ap()))
                xb = const.tile([128, NCH, D], f32)
                for c in range(NCH):
                    nc.any.tensor_add(xb[:, c, :], x_sb[:, c, :], bias_bc)

        for c in range(NCH):
            y_chunk(c)

    nc.compile()
    return nc


def _get_program():
    global _PROGRAM
    if _PROGRAM is None:
        _PROGRAM = _build_program()
    return _PROGRAM


def _make_in_maps(inputs):
    full = {k: np.asarray(v) for k, v in inputs.items()}
    in_maps = []
    for b in range(8):
        mb = np.asarray(full["mask"][b], dtype=np.int32)
        idx = np.nonzero(mb)[0]
        if len(idx) > KP:
            raise ValueError(f"unmasked key count {len(idx)} exceeds KP={KP}")
        idx_pad = np.zeros(KP, dtype=np.int64)
        idx_pad[: len(idx)] = idx
        mk = np.zeros(KP, dtype=np.int32)
        mk[: len(idx)] = 1
        xb_full = np.asarray(full["x"][b], dtype=np.float32)
        m = {
            "x": np.ascontiguousarray(xb_full),
            "mask": np.ascontiguousarray(mb),
            "x_keys": np.ascontiguousarray(xb_full[idx_pad]),
            "mask_keys": mk,
        }
        for k in ("wq", "wk", "wv", "wg", "out_w", "out_b", "ln_g", "ln_b",
                  "lnr_g", "lnr_b", "lno_g", "lno_b"):
            m[k] = np.ascontiguousarray(full[k], dtype=np.float32)
        in_maps.append(m)
    return in_maps


def run_on_hw(inputs, trace=False):
    """Run on the 8 NeuronCores; returns (output [8,1024,256] f32, results obj)."""
    from concourse import bass_utils

    nc = _get_program()
    in_maps = _make_in_maps(inputs)
    res = bass_utils.run_bass_kernel_spmd(
        nc, in_maps, core_ids=list(range(8)), trace=trace
    )
    out = np.stack([res.results[b]["y"] for b in range(8)], axis=0).astype(np.float32)
    return out, res


def _run_sim(inputs):
    """CoreSim fallback (slow but exact): used only if hardware runs fail."""
    from concourse.bass_interp import CoreSim

    nc = _get_program()
    in_maps = _make_in_maps(inputs)
    outs = []
    for b in range(8):
        sim = CoreSim(nc, trace=False)
        for name, val in in_maps[b].items():
            sim.tensor(name)[:] = val
        sim.simulate(check_with_hw=False)
        outs.append(sim.tensor("y").copy())
    return np.stack(outs, axis=0).astype(np.float32)


def kernel(**inputs) -> np.ndarray:
    last_err = None
    for _ in range(3):
        try:
            out, _ = run_on_hw(inputs, trace=False)
        except Exception as e:  # transient PJRT/compile hiccups: retry
            last_err = e
            continue
        if np.isfinite(out).all():
            return out
    try:
        return _run_sim(inputs)
    except Exception:
        if last_err is not None:
            raise last_err
        raise
